# revision 31
# baseline (speedup 1.0000x reference)
"""Trainium2 Bass kernel for nn_Attention_26173530702697.

Dense transformer block (sigmoid attention x2, PEG depthwise conv, LN x3,
MLP) on decoder [8, 384, 32, 32]. Sharding: pure data parallel over batch
(B=8 == 8 cores), zero collectives. Everything on a core stays d-major
[384, 1024] (channels on partitions), which makes the PEG conv and all
per-channel affine ops per-partition, and feeds the matmuls directly.

Matmul operands are bf16 (1 cycle/row on the PE); accumulation is fp32 in
PSUM; the residual / PEG / LN chain stays fp32 on the vector engine.
"""

import math
import os

import ml_dtypes
import numpy as np

import concourse.bass as bass
import concourse.tile as tile
from concourse import bacc
from concourse import mybir
from concourse.bass_utils import run_bass_kernel_spmd

F32 = mybir.dt.float32
BF16 = mybir.dt.bfloat16
AF = mybir.ActivationFunctionType
OP = mybir.AluOpType

B, DIM, H, W = 8, 384, 32, 32
HEADS, DK = 8, 96
N = H * W            # 1024
C3 = DIM // 128      # 3 channel tiles
H6 = 768 // 128      # 6 hidden tiles
EPS = 1e-5
HALF = 512

LAST_EXEC_TIME_NS = None


def build_nc():
    nc = bacc.Bacc("TRN2", target_bir_lowering=False, debug=False,
                   enable_asserts=True, num_devices=B)

    def _param(name, shape, dt=BF16, out=False):
        return nc.dram_tensor(name, shape, dt,
                              kind="ExternalOutput" if out else "ExternalInput").ap()

    # ---- DRAM parameters (per-core shapes; weights replicated) ----
    x_ext = _param("x", [128, C3, N])
    out_ext = _param("out", [C3, 128, N], F32, out=True)

    wq_ext, wk_ext, wv_ext = {}, {}, {}
    bv_ext = {}
    for i in (1, 2):
        wq_ext[i] = _param(f"wq{i}", [HEADS, 128, C3, DK])
        wk_ext[i] = _param(f"wk{i}", [HEADS, 128, C3, DK])
        wv_ext[i] = _param(f"wv{i}", [HEADS, 128, C3, DIM])
        bv_ext[i] = _param(f"bv{i}", [HEADS, DIM])
    constf_ext = _param("constf", [128, 96], F32)
    constg_ext = _param("constg", [1, 3 * DIM])
    w1_ext = _param("mlp_w1", [128, C3, 768])
    w2_ext = _param("mlp_w2", [128, H6, DIM])

    MM = nc.tensor.matmul

    with tile.TileContext(nc) as tc:
        with (
            tc.tile_pool(name="xp", bufs=12) as xp,
            tc.tile_pool(name="xb", bufs=12) as xb,        # bf16 shadows / LN outs
            tc.tile_pool(name="stat", bufs=6) as stat,
            tc.tile_pool(name="const", bufs=1) as constp,
            tc.tile_pool(name="ps", bufs=4, space="PSUM") as psp,
        ):
            # ---- input + first head weights lead the DMA queue ----
            xin = constp.tile([128, C3, N], BF16, name="xin", tag="xin")
            nc.sync.dma_start(xin[:], x_ext[:])
            pf_wq = constp.tile([128, C3, DK], BF16, name="pf_wq", tag="pf_wq")
            nc.sync.dma_start(pf_wq[:], wq_ext[1][0])
            pf_wk = constp.tile([128, C3, DK], BF16, name="pf_wk", tag="pf_wk")
            nc.sync.dma_start(pf_wk[:], wk_ext[1][0])
            pf0 = (pf_wq, pf_wk)

            # ---- constants ----
            ones_col = constp.tile([128, 1], BF16, name="ones_col", tag="ones_col")
            nc.vector.memset(ones_col[:], 1.0)
            ones_row = constp.tile([1, 128], BF16, name="ones_row", tag="ones_row")
            nc.vector.memset(ones_row[:], 1.0)
            inv_col = constp.tile([128, 1], BF16, name="inv_col", tag="inv_col")
            nc.vector.memset(inv_col[:], 1.0 / DIM)
            eps_t = constp.tile([1, 1], F32, name="eps_t", tag="eps_t")
            nc.vector.memset(eps_t[:], EPS)

            cf = constp.tile([128, 96], F32, name="cf", tag="cf")
            nc.sync.dma_start(cf[:], constf_ext[:])
            cg = constp.tile([1, 3 * DIM], BF16, name="cg", tag="cg")
            nc.sync.dma_start(cg[:], constg_ext[:])
            # packed fp32 const columns (see _prep_weights)
            bet = {k: cf[:, 3 * j:3 * j + 3]
                   for j, k in enumerate(("ln1", "mlpln", "ln2"))}
            gam = {k: cg[:, j * DIM:(j + 1) * DIM]
                   for j, k in enumerate(("ln1", "mlpln", "ln2"))}
            a_sb = {1: cf[:, 9:12], 2: cf[:, 12:15]}
            a3_sb = cf[:, 15:18]
            bp_sb = {1: cf[:, 18:21], 2: cf[:, 21:24]}
            pegw_sb = cf[:, 24:51].rearrange("p (c t) -> p c t", t=9)
            pegb_sb = cf[:, 51:54]
            b1_sb = cf[:, 54:60]
            b2_sb = cf[:, 60:63]
            bq_sb = {1: cf[0:DK, 63:71], 2: cf[0:DK, 79:87]}
            bk_sb = {1: cf[0:DK, 71:79], 2: cf[0:DK, 87:95]}

            def layer_norm(x_tiles, key, out_dt, out_pool):
                """LN over channel axis (partitions). Colsums with a 1/DIM
                weight column give mu and E[x^2] directly; rsqrt via
                exp(-0.5*ln(var+eps)); normalize via rank-1 broadcasts.
                """
                g_row, b_col = gam[key], bet[key]
                mu_ps = psp.tile([1, N], F32, name="mu_ps", tag="ps")
                ex2_ps = psp.tile([1, N], F32, name="ex2_ps", tag="ps")
                for c in range(C3):
                    if x_tiles[c].dtype == BF16:
                        xsc = x_tiles[c]
                    else:
                        xsc = xb.tile([128, N], BF16, name="xs", tag="xb")
                        nc.scalar.copy(xsc[:], x_tiles[c][:])
                    s = xb.tile([128, N], BF16, name="sq", tag="xb")
                    nc.scalar.square(s[:], x_tiles[c][:])
                    for hlf in range(2):
                        sl = slice(hlf * HALF, (hlf + 1) * HALF)
                        MM(mu_ps[:, sl], inv_col[:], xsc[:, sl],
                           start=(c == 0), stop=(c == C3 - 1))
                        MM(ex2_ps[:, sl], inv_col[:], s[:, sl],
                           start=(c == 0), stop=(c == C3 - 1))
                mu = stat.tile([1, N], F32, name="mu", tag="stat")
                nc.vector.tensor_copy(mu[:], mu_ps[:])
                mu2 = stat.tile([1, N], F32, name="mu2", tag="stat")
                nc.scalar.square(mu2[:], mu_ps[:])
                var = stat.tile([1, N], F32, name="var", tag="stat")
                nc.vector.scalar_tensor_tensor(
                    var[:], ex2_ps[:], 1.0, mu2[:],
                    op0=OP.mult, op1=OP.subtract)
                rstd = stat.tile([1, N], BF16, name="rstd", tag="stat")
                nc.scalar.activation(rstd[:], var[:], AF.Abs_reciprocal_sqrt,
                                     bias=eps_t[:])
                mc = stat.tile([1, N], BF16, name="mc", tag="stat")
                nc.vector.tensor_mul(mc[:], mu[:], rstd[:])
                A, Cg = [], []
                for c in range(C3):
                    g_seg = g_row[:, c * 128:(c + 1) * 128]
                    Ac = psp.tile([128, N], F32, name="A", tag="ps")
                    for hlf in range(2):
                        sl = slice(hlf * HALF, (hlf + 1) * HALF)
                        MM(Ac[:, sl], g_seg, rstd[:, sl], start=True, stop=True)
                    A.append(Ac)
                for c in range(C3):
                    g_seg = g_row[:, c * 128:(c + 1) * 128]
                    Cc = psp.tile([128, N], F32, name="Cg", tag="ps")
                    for hlf in range(2):
                        sl = slice(hlf * HALF, (hlf + 1) * HALF)
                        MM(Cc[:, sl], g_seg, mc[:, sl], start=True, stop=True)
                    Cg.append(Cc)
                out = []
                for c in range(C3):
                    t1 = xp.tile([128, N], F32, name="t1", tag="x")
                    nc.vector.tensor_mul(t1[:], x_tiles[c][:], A[c][:])
                    y = out_pool.tile([128, N], out_dt, name="lnout",
                                      tag="x" if out_pool is xp else "xb")
                    nc.vector.scalar_tensor_tensor(
                        y[:], t1[:], b_col[:, c:c + 1], Cg[c][:],
                        op0=OP.add, op1=OP.subtract)
                    out.append(y)
                return out

            def mha(i, x_tiles, pools, prefetch=None):
                """y = a_i * x + MHA_i(x); x_tiles bf16 d-major; returns fp32.

                Head loop is software-pipelined: head h's O/projector matmuls
                are emitted after head h+1's QKV/score matmuls so the PE
                stream covers the sigmoid latency of head h+1.
                """
                wq_p, wv_p, st_p, v_p, qk_p, bvb_p = pools
                Y = []
                for c in range(C3):
                    y = xp.tile([128, N], F32, name="yres", tag="x")
                    nc.vector.tensor_scalar(
                        y[:], x_tiles[c][:], a_sb[i][:, c:c + 1], bp_sb[i][:, c:c + 1],
                        op0=OP.mult, op1=OP.add)
                    Y.append(y)

                def qkvst(h):
                    if h == 0 and prefetch is not None:
                        wq_t, wk_t = prefetch
                    else:
                        wq_t = wq_p.tile([128, C3, DK], BF16, name="wq", tag="wq")
                        nc.sync.dma_start(wq_t[:], wq_ext[i][h])
                        wk_t = wq_p.tile([128, C3, DK], BF16, name="wk", tag="wk")
                        nc.sync.dma_start(wk_t[:], wk_ext[i][h])
                    wv_t = wv_p.tile([128, C3, DIM], BF16, name="wv", tag="wv")
                    nc.sync.dma_start(wv_t[:], wv_ext[i][h])
                    bv_row = bvb_p.tile([1, DIM], BF16, name="bvrow", tag="bvrow")
                    nc.sync.dma_start(bv_row[:], bv_ext[i][h].unsqueeze(0))

                    # Q^T, K^T: [96, 1024] d-major (score scale folded into wq)
                    qt_ps = psp.tile([DK, N], F32, name="qt_ps", tag="ps")
                    kt_ps = psp.tile([DK, N], F32, name="kt_ps", tag="ps")
                    qt = qk_p.tile([DK, N], BF16, name="qt", tag="qk")
                    kt = qk_p.tile([DK, N], BF16, name="kt", tag="qk")
                    for hlf in range(2):
                        sl = slice(hlf * HALF, (hlf + 1) * HALF)
                        for c in range(C3):
                            MM(kt_ps[:, sl], wk_t[:, c, :], x_tiles[c][:, sl],
                               start=(c == 0), stop=(c == C3 - 1))
                    for hlf in range(2):
                        sl = slice(hlf * HALF, (hlf + 1) * HALF)
                        for c in range(C3):
                            MM(qt_ps[:, sl], wq_t[:, c, :], x_tiles[c][:, sl],
                               start=(c == 0), stop=(c == C3 - 1))
                        nc.vector.tensor_scalar_add(
                            kt[:, sl], kt_ps[:, sl], bk_sb[i][:, h:h + 1])
                        nc.vector.tensor_scalar_add(
                            qt[:, sl], qt_ps[:, sl], bq_sb[i][:, h:h + 1])

                    bvb_ps = psp.tile([128, DIM], F32, name="bvb_ps", tag="ps")
                    MM(bvb_ps[:], ones_row[:], bv_row[:], start=True, stop=True)
                    bvb = bvb_p.tile([128, DIM], BF16, name="bvb", tag="bvb")
                    nc.vector.tensor_copy(bvb[:], bvb_ps[:])

                    # interleave V and S^T so V matmuls cover sigmoid latency
                    v_sb, st_sb = [], []
                    for kc in range(HEADS):
                        ksl = slice(kc * 128, (kc + 1) * 128)
                        v_ps = psp.tile([128, DIM], F32, name="v_ps", tag="ps")
                        for c in range(C3):
                            MM(v_ps[:], x_tiles[c][:, ksl], wv_t[:, c, :],
                               start=(c == 0), stop=(c == C3 - 1))
                        v = v_p.tile([128, DIM], BF16, name="v", tag="v")
                        nc.vector.tensor_add(v[:], v_ps[:], bvb[:])
                        v_sb.append(v)
                        st_ps = psp.tile([128, N], F32, name="st_ps", tag="ps")
                        for hlf in range(2):
                            sl = slice(hlf * HALF, (hlf + 1) * HALF)
                            MM(st_ps[:, sl], kt[:, ksl], qt[:, sl],
                               start=True, stop=True)
                        s = st_p.tile([128, N], BF16, name="s", tag="st")
                        nc.scalar.activation(s[:], st_ps[:], AF.Sigmoid)
                        st_sb.append(s)
                    return v_sb, st_sb

                def oproj(state):
                    # wp is folded into wv on the host, so the score-value
                    # product lands directly in output-channel space.
                    v_sb, st_sb = state
                    for dm in range(C3):
                        dsl = slice(dm * 128, (dm + 1) * 128)
                        o_ps = psp.tile([128, N], F32, name="o_ps", tag="ps")
                        for hlf in range(2):
                            sl = slice(hlf * HALF, (hlf + 1) * HALF)
                            for kc in range(HEADS):
                                MM(o_ps[:, sl], v_sb[kc][:, dsl], st_sb[kc][:, sl],
                                   start=(kc == 0), stop=(kc == HEADS - 1))
                        nc.vector.tensor_add(Y[dm][:], o_ps[:], Y[dm][:])

                state = qkvst(0)
                for h in range(1, HEADS):
                    nxt = qkvst(h)
                    oproj(state)
                    state = nxt
                oproj(state)
                return Y

            def peg(x_tiles):
                """Depthwise 3x3 SAME conv + bias (fp32 in/out)."""
                out = []
                for c in range(C3):
                    acc = xp.tile([128, N], F32, name="peg_acc", tag="x")
                    nc.scalar.activation(
                        acc[:], x_tiles[c][:], AF.Identity,
                        bias=pegb_sb[:, c:c + 1], scale=pegw_sb[:, c, 4:5])
                    a3d = acc[:].rearrange("p (h w) -> p h w", w=W)
                    x3d = x_tiles[c][:].rearrange("p (h w) -> p h w", w=W)
                    eng = nc.vector
                    for dy in (-1, 0, 1):
                        for dx in (-1, 0, 1):
                            if dy == 0 and dx == 0:
                                continue
                            tap = 3 * (dy + 1) + (dx + 1)
                            oh = slice(max(0, -dy), H - max(0, dy))
                            ow = slice(max(0, -dx), W - max(0, dx))
                            ih = slice(max(0, dy), H + min(0, dy))
                            iw = slice(max(0, dx), W + min(0, dx))
                            eng.scalar_tensor_tensor(
                                a3d[:, oh, ow], x3d[:, ih, iw],
                                pegw_sb[:, c, tap:tap + 1], a3d[:, oh, ow],
                                op0=OP.mult, op1=OP.add)
                    out.append(acc)
                return out

            x0 = [xin[:, c, :] for c in range(C3)]

            with (
                tc.tile_pool(name="wq", bufs=4) as wq_p,
                tc.tile_pool(name="wv", bufs=3) as wv_p,
                tc.tile_pool(name="st", bufs=20) as st_p,
                tc.tile_pool(name="v", bufs=20) as v_p,
                tc.tile_pool(name="qk", bufs=6) as qk_p,
                tc.tile_pool(name="bvb", bufs=2) as bvb_p,
            ):
                pools = (wq_p, wv_p, st_p, v_p, qk_p, bvb_p)
                x1 = mha(1, x0, pools, prefetch=pf0)
                x2 = peg(x1)
                x3 = layer_norm(x2, "ln1", BF16, xb)
                x4 = mha(2, x3, pools)

            with tc.tile_pool(name="mlp", bufs=1) as mlp_p, \
                 tc.tile_pool(name="hid", bufs=8) as hid_p:
                w1_sb = mlp_p.tile([128, C3, 768], BF16, name="w1", tag="w1")
                nc.sync.dma_start(w1_sb[:], w1_ext[:])
                w2_sb = mlp_p.tile([128, H6, DIM], BF16, name="w2", tag="w2")
                nc.sync.dma_start(w2_sb[:], w2_ext[:])

                hn = layer_norm(x4, "mlpln", BF16, xb)
                u_sb = []
                for dm in range(C3):
                    u = xp.tile([128, N], F32, name="u", tag="x")
                    nc.vector.tensor_scalar(
                        u[:], x4[dm][:], a3_sb[:, dm:dm + 1], b2_sb[:, dm:dm + 1],
                        op0=OP.mult, op1=OP.add)
                    u_sb.append(u)
                hid = []
                for ht in range(H6):
                    hsl = slice(ht * 128, (ht + 1) * 128)
                    hd_ps = psp.tile([128, N], F32, name="hd_ps", tag="ps")
                    for hlf in range(2):
                        sl = slice(hlf * HALF, (hlf + 1) * HALF)
                        for c in range(C3):
                            MM(hd_ps[:, sl], w1_sb[:, c, hsl], hn[c][:, sl],
                               start=(c == 0), stop=(c == C3 - 1))
                    hg = hid_p.tile([128, N], BF16, name="hg", tag="hid")
                    nc.scalar.activation(hg[:], hd_ps[:], AF.Gelu,
                                         bias=b1_sb[:, ht:ht + 1])
                    hid.append(hg)
                x5 = []
                for dm in range(C3):
                    dsl = slice(dm * 128, (dm + 1) * 128)
                    o2_ps = psp.tile([128, N], F32, name="o2_ps", tag="ps")
                    for hlf in range(2):
                        sl = slice(hlf * HALF, (hlf + 1) * HALF)
                        for ht in range(H6):
                            MM(o2_ps[:, sl], w2_sb[:, ht, dsl], hid[ht][:, sl],
                               start=(ht == 0), stop=(ht == H6 - 1))
                    y = xp.tile([128, N], F32, name="x5t", tag="x")
                    nc.vector.tensor_add(y[:], o2_ps[:], u_sb[dm][:])
                    x5.append(y)

                yout = layer_norm(x5, "ln2", F32, xp)
                for c in range(C3):
                    nc.sync.dma_start(out_ext[c], yout[c][:])

    nc.compile()
    return nc


def _prep_weights(inputs):
    """Host-side reshapes into SBUF-tile-friendly layouts."""
    g = {k: np.ascontiguousarray(np.asarray(v, dtype=np.float32))
         for k, v in inputs.items()}
    s = 1.0 / math.sqrt(DK)
    bf = ml_dtypes.bfloat16
    m = {}
    for i in (1, 2):
        wq = g[f"wq{i}"] * s                      # fold score scale into Q
        m[f"wq{i}"] = wq.reshape(HEADS, C3, 128, DK).transpose(0, 2, 1, 3).astype(bf)
        m[f"wk{i}"] = g[f"wk{i}"].reshape(HEADS, C3, 128, DK).transpose(0, 2, 1, 3).astype(bf)
        wp = g[f"wp{i}"].reshape(HEADS, DIM, DIM)          # [h, 384, 384]
        wvp = np.einsum("hdf,hfe->hde", g[f"wv{i}"], wp)   # fold projector
        bvp = np.einsum("hf,hfe->he", g[f"bv{i}"], wp)
        m[f"wv{i}"] = wvp.reshape(HEADS, C3, 128, DIM).transpose(0, 2, 1, 3).astype(bf)
        m[f"bv{i}"] = bvp.astype(bf)              # [8, 384]
    m["mlp_w1"] = g["mlp_w1"].reshape(C3, 128, 768).transpose(1, 0, 2).astype(bf)
    m["mlp_w2"] = g["mlp_w2"].reshape(H6, 128, DIM).transpose(1, 0, 2).astype(bf)

    def col3(v):
        return np.asarray(v, np.float32).reshape(DIM).reshape(C3, 128).T

    cf = np.zeros((128, 96), np.float32)
    for j, k in enumerate(("ln1", "mlpln", "ln2")):
        cf[:, 3 * j:3 * j + 3] = col3(g[f"{k}_b"])
    cf[:, 9:12] = col3(g["a1"]); cf[:, 12:15] = col3(g["a2"])
    cf[:, 15:18] = col3(g["a3"])
    cf[:, 18:21] = col3(g["bp1"]); cf[:, 21:24] = col3(g["bp2"])
    cf[:, 24:51] = g["peg_w"].reshape(DIM, 9).reshape(C3, 128, 9).transpose(
        1, 0, 2).reshape(128, 27)
    cf[:, 51:54] = col3(g["peg_b"])
    cf[:, 54:60] = g["mlp_b1"].reshape(H6, 128).T
    cf[:, 60:63] = col3(g["mlp_b2"])
    cf[0:DK, 63:71] = (g["bq1"] * s).T
    cf[0:DK, 71:79] = g["bk1"].T
    cf[0:DK, 79:87] = (g["bq2"] * s).T
    cf[0:DK, 87:95] = g["bk2"].T
    m["constf"] = cf
    cg = np.concatenate([g[f"{k}_g"].reshape(DIM)
                         for k in ("ln1", "mlpln", "ln2")]).reshape(1, 3 * DIM)
    m["constg"] = cg.astype(bf)
    m = {k: np.ascontiguousarray(v) for k, v in m.items()}
    return m, g


_NC_CACHE = None


def kernel(**inputs) -> np.ndarray:
    global LAST_EXEC_TIME_NS, _NC_CACHE
    weights, g = _prep_weights(inputs)
    bf = ml_dtypes.bfloat16
    dec = g["decoder"].reshape(B, C3, 128, N).transpose(0, 2, 1, 3).astype(bf)

    if _NC_CACHE is None:
        _NC_CACHE = build_nc()
    nc = _NC_CACHE

    in_maps = []
    for b in range(B):
        im = {"x": np.ascontiguousarray(dec[b])}
        im.update(weights)
        in_maps.append(im)

    trace = bool(int(os.environ.get("KERNEL_TRACE", "0")))
    if trace:
        trace = _install_profile_hook()
    res = run_bass_kernel_spmd(nc, in_maps, core_ids=list(range(B)), trace=trace)
    LAST_EXEC_TIME_NS = res.exec_time_ns

    out = np.stack([np.asarray(res.results[b]["out"]) for b in range(B)], axis=0)
    return np.ascontiguousarray(
        out.reshape(B, DIM, H, W).astype(np.float32))


def _install_profile_hook():
    """Register the axon NTFF profiling hook this image's antenv lacks."""
    import sys
    import types
    try:
        from concourse import bass_utils as _bu
        _bu.upload_artifacts = lambda tmpdir: tmpdir
        try:
            import antenv.axon_hooks  # noqa: F401
            return True
        except ImportError:
            pass
        import antenv
        mod = types.ModuleType("antenv.axon_hooks")
        state = {"hook": None}
        mod.set_axon_ntff_profile_hook = lambda h: state.__setitem__("hook", h)
        mod.get_axon_ntff_profile_hook = lambda: state["hook"]
        sys.modules["antenv.axon_hooks"] = mod
        antenv.axon_hooks = mod
        from trn_agent_boot.trn_boot import _ntff_profile_via_ctypes
        mod.set_axon_ntff_profile_hook(
            _ntff_profile_via_ctypes("/opt/axon/libaxon_pjrt.so"))
        return True
    except Exception:
        return False


# revision 32
# speedup vs baseline: 1.0031x; 1.0031x over previous
"""Trainium2 Bass kernel for nn_Attention_26173530702697.

Dense transformer block (sigmoid attention x2, PEG depthwise conv, LN x3,
MLP) on decoder [8, 384, 32, 32]. Sharding: pure data parallel over batch
(B=8 == 8 cores), zero collectives. Everything on a core stays d-major
[384, 1024] (channels on partitions), which makes the PEG conv and all
per-channel affine ops per-partition, and feeds the matmuls directly.

Matmul operands are bf16 (1 cycle/row on the PE); accumulation is fp32 in
PSUM; the residual / PEG / LN chain stays fp32 on the vector engine.
"""

import math
import os

import ml_dtypes
import numpy as np

import concourse.bass as bass
import concourse.tile as tile
from concourse import bacc
from concourse import mybir
from concourse.bass_utils import run_bass_kernel_spmd

F32 = mybir.dt.float32
BF16 = mybir.dt.bfloat16
AF = mybir.ActivationFunctionType
OP = mybir.AluOpType

B, DIM, H, W = 8, 384, 32, 32
HEADS, DK = 8, 96
N = H * W            # 1024
C3 = DIM // 128      # 3 channel tiles
H6 = 768 // 128      # 6 hidden tiles
EPS = 1e-5
HALF = 512

LAST_EXEC_TIME_NS = None


def build_nc():
    nc = bacc.Bacc("TRN2", target_bir_lowering=False, debug=False,
                   enable_asserts=True, num_devices=B)

    def _param(name, shape, dt=BF16, out=False):
        return nc.dram_tensor(name, shape, dt,
                              kind="ExternalOutput" if out else "ExternalInput").ap()

    # ---- DRAM parameters (per-core shapes; weights replicated) ----
    x_ext = _param("x", [128, C3, N])
    out_ext = _param("out", [C3, 128, N], F32, out=True)

    wq_ext, wk_ext, wv_ext = {}, {}, {}
    bv_ext = {}
    for i in (1, 2):
        wq_ext[i] = _param(f"wq{i}", [HEADS, 128, C3, DK])
        wk_ext[i] = _param(f"wk{i}", [HEADS, 128, C3, DK])
        wv_ext[i] = _param(f"wv{i}", [HEADS, 128, C3, DIM])
        bv_ext[i] = _param(f"bv{i}", [HEADS, DIM])
    constf_ext = _param("constf", [128, 96], F32)
    constg_ext = _param("constg", [1, 3 * DIM])
    w1_ext = _param("mlp_w1", [128, C3, 768])
    w2_ext = _param("mlp_w2", [128, H6, DIM])

    MM = nc.tensor.matmul

    with tile.TileContext(nc) as tc:
        with (
            tc.tile_pool(name="xp", bufs=12) as xp,
            tc.tile_pool(name="xb", bufs=12) as xb,        # bf16 shadows / LN outs
            tc.tile_pool(name="stat", bufs=5) as stat,
            tc.tile_pool(name="const", bufs=1) as constp,
            tc.tile_pool(name="ps", bufs=4, space="PSUM") as psp,
        ):
            # ---- input first so its DMA leads the queue ----
            xin = constp.tile([128, C3, N], BF16, name="xin", tag="xin")
            nc.sync.dma_start(xin[:], x_ext[:])

            # ---- constants ----
            ones_col = constp.tile([128, 1], BF16, name="ones_col", tag="ones_col")
            nc.vector.memset(ones_col[:], 1.0)
            ones_row = constp.tile([1, 128], BF16, name="ones_row", tag="ones_row")
            nc.vector.memset(ones_row[:], 1.0)
            inv_col = constp.tile([128, 1], BF16, name="inv_col", tag="inv_col")
            nc.vector.memset(inv_col[:], 1.0 / DIM)
            eps_t = constp.tile([1, 1], F32, name="eps_t", tag="eps_t")
            nc.vector.memset(eps_t[:], EPS)

            cf = constp.tile([128, 96], F32, name="cf", tag="cf")
            nc.sync.dma_start(cf[:], constf_ext[:])
            cg = constp.tile([1, 3 * DIM], BF16, name="cg", tag="cg")
            nc.sync.dma_start(cg[:], constg_ext[:])
            # packed fp32 const columns (see _prep_weights)
            bet = {k: cf[:, 3 * j:3 * j + 3]
                   for j, k in enumerate(("ln1", "mlpln", "ln2"))}
            gam = {k: cg[:, j * DIM:(j + 1) * DIM]
                   for j, k in enumerate(("ln1", "mlpln", "ln2"))}
            a_sb = {1: cf[:, 9:12], 2: cf[:, 12:15]}
            a3_sb = cf[:, 15:18]
            bp_sb = {1: cf[:, 18:21], 2: cf[:, 21:24]}
            pegw_sb = cf[:, 24:51].rearrange("p (c t) -> p c t", t=9)
            pegb_sb = cf[:, 51:54]
            b1_sb = cf[:, 54:60]
            b2_sb = cf[:, 60:63]
            bq_sb = {1: cf[0:DK, 63:71], 2: cf[0:DK, 79:87]}
            bk_sb = {1: cf[0:DK, 71:79], 2: cf[0:DK, 87:95]}

            def layer_norm(x_tiles, key, out_dt, out_pool):
                """LN over channel axis (partitions). Colsums with a 1/DIM
                weight column give mu and E[x^2] directly; rsqrt via
                exp(-0.5*ln(var+eps)); normalize via rank-1 broadcasts.
                """
                g_row, b_col = gam[key], bet[key]
                mu_ps = psp.tile([1, N], F32, name="mu_ps", tag="ps")
                ex2_ps = psp.tile([1, N], F32, name="ex2_ps", tag="ps")
                for c in range(C3):
                    if x_tiles[c].dtype == BF16:
                        xsc = x_tiles[c]
                    else:
                        xsc = xb.tile([128, N], BF16, name="xs", tag="xb")
                        nc.scalar.copy(xsc[:], x_tiles[c][:])
                    s = xb.tile([128, N], BF16, name="sq", tag="xb")
                    nc.scalar.square(s[:], x_tiles[c][:])
                    for hlf in range(2):
                        sl = slice(hlf * HALF, (hlf + 1) * HALF)
                        MM(mu_ps[:, sl], inv_col[:], xsc[:, sl],
                           start=(c == 0), stop=(c == C3 - 1))
                        MM(ex2_ps[:, sl], inv_col[:], s[:, sl],
                           start=(c == 0), stop=(c == C3 - 1))
                mu = stat.tile([1, N], F32, name="mu", tag="stat")
                nc.vector.tensor_copy(mu[:], mu_ps[:])
                mu2 = stat.tile([1, N], F32, name="mu2", tag="stat")
                nc.scalar.square(mu2[:], mu_ps[:])
                var = stat.tile([1, N], F32, name="var", tag="stat")
                nc.vector.scalar_tensor_tensor(
                    var[:], ex2_ps[:], 1.0, mu2[:],
                    op0=OP.mult, op1=OP.subtract)
                rstd = stat.tile([1, N], BF16, name="rstd", tag="stat")
                nc.scalar.activation(rstd[:], var[:], AF.Abs_reciprocal_sqrt,
                                     bias=eps_t[:])
                mc = stat.tile([1, N], BF16, name="mc", tag="stat")
                nc.vector.tensor_mul(mc[:], mu[:], rstd[:])
                A, Cg = [], []
                for c in range(C3):
                    g_seg = g_row[:, c * 128:(c + 1) * 128]
                    Ac = psp.tile([128, N], F32, name="A", tag="ps")
                    for hlf in range(2):
                        sl = slice(hlf * HALF, (hlf + 1) * HALF)
                        MM(Ac[:, sl], g_seg, rstd[:, sl], start=True, stop=True)
                    A.append(Ac)
                for c in range(C3):
                    g_seg = g_row[:, c * 128:(c + 1) * 128]
                    Cc = psp.tile([128, N], F32, name="Cg", tag="ps")
                    for hlf in range(2):
                        sl = slice(hlf * HALF, (hlf + 1) * HALF)
                        MM(Cc[:, sl], g_seg, mc[:, sl], start=True, stop=True)
                    Cg.append(Cc)
                out = []
                for c in range(C3):
                    t1 = xp.tile([128, N], F32, name="t1", tag="x")
                    nc.vector.tensor_mul(t1[:], x_tiles[c][:], A[c][:])
                    y = out_pool.tile([128, N], out_dt, name="lnout",
                                      tag="x" if out_pool is xp else "xb")
                    nc.vector.scalar_tensor_tensor(
                        y[:], t1[:], b_col[:, c:c + 1], Cg[c][:],
                        op0=OP.add, op1=OP.subtract)
                    out.append(y)
                return out

            def mha(i, x_tiles, pools):
                """y = a_i * x + MHA_i(x); x_tiles bf16 d-major; returns fp32.

                Head loop is software-pipelined: head h's O/projector matmuls
                are emitted after head h+1's QKV/score matmuls so the PE
                stream covers the sigmoid latency of head h+1.
                """
                wq_p, wv_p, st_p, v_p, qk_p, bvb_p = pools
                Y = []
                for c in range(C3):
                    y = xp.tile([128, N], F32, name="yres", tag="x")
                    nc.vector.tensor_scalar(
                        y[:], x_tiles[c][:], a_sb[i][:, c:c + 1], bp_sb[i][:, c:c + 1],
                        op0=OP.mult, op1=OP.add)
                    Y.append(y)

                def qkvst(h):
                    wq_t = wq_p.tile([128, C3, DK], BF16, name="wq", tag="wq")
                    nc.sync.dma_start(wq_t[:], wq_ext[i][h])
                    wk_t = wq_p.tile([128, C3, DK], BF16, name="wk", tag="wk")
                    nc.sync.dma_start(wk_t[:], wk_ext[i][h])
                    wv_t = wv_p.tile([128, C3, DIM], BF16, name="wv", tag="wv")
                    nc.sync.dma_start(wv_t[:], wv_ext[i][h])
                    bv_row = bvb_p.tile([1, DIM], BF16, name="bvrow", tag="bvrow")
                    nc.sync.dma_start(bv_row[:], bv_ext[i][h].unsqueeze(0))

                    # Q^T, K^T: [96, 1024] d-major (score scale folded into wq)
                    qt_ps = psp.tile([DK, N], F32, name="qt_ps", tag="ps")
                    kt_ps = psp.tile([DK, N], F32, name="kt_ps", tag="ps")
                    qt = qk_p.tile([DK, N], BF16, name="qt", tag="qk")
                    kt = qk_p.tile([DK, N], BF16, name="kt", tag="qk")
                    for hlf in range(2):
                        sl = slice(hlf * HALF, (hlf + 1) * HALF)
                        for c in range(C3):
                            MM(kt_ps[:, sl], wk_t[:, c, :], x_tiles[c][:, sl],
                               start=(c == 0), stop=(c == C3 - 1))
                    for hlf in range(2):
                        sl = slice(hlf * HALF, (hlf + 1) * HALF)
                        for c in range(C3):
                            MM(qt_ps[:, sl], wq_t[:, c, :], x_tiles[c][:, sl],
                               start=(c == 0), stop=(c == C3 - 1))
                        nc.vector.tensor_scalar_add(
                            kt[:, sl], kt_ps[:, sl], bk_sb[i][:, h:h + 1])
                        nc.vector.tensor_scalar_add(
                            qt[:, sl], qt_ps[:, sl], bq_sb[i][:, h:h + 1])

                    bvb_ps = psp.tile([128, DIM], F32, name="bvb_ps", tag="ps")
                    MM(bvb_ps[:], ones_row[:], bv_row[:], start=True, stop=True)
                    bvb = bvb_p.tile([128, DIM], BF16, name="bvb", tag="bvb")
                    nc.vector.tensor_copy(bvb[:], bvb_ps[:])

                    # interleave V and S^T so V matmuls cover sigmoid latency
                    v_sb, st_sb = [], []
                    for kc in range(HEADS):
                        ksl = slice(kc * 128, (kc + 1) * 128)
                        v_ps = psp.tile([128, DIM], F32, name="v_ps", tag="ps")
                        for c in range(C3):
                            MM(v_ps[:], x_tiles[c][:, ksl], wv_t[:, c, :],
                               start=(c == 0), stop=(c == C3 - 1))
                        v = v_p.tile([128, DIM], BF16, name="v", tag="v")
                        nc.vector.tensor_add(v[:], v_ps[:], bvb[:])
                        v_sb.append(v)
                        st_ps = psp.tile([128, N], F32, name="st_ps", tag="ps")
                        for hlf in range(2):
                            sl = slice(hlf * HALF, (hlf + 1) * HALF)
                            MM(st_ps[:, sl], kt[:, ksl], qt[:, sl],
                               start=True, stop=True)
                        s = st_p.tile([128, N], BF16, name="s", tag="st")
                        nc.scalar.activation(s[:], st_ps[:], AF.Sigmoid)
                        st_sb.append(s)
                    return v_sb, st_sb

                def oproj(state):
                    # wp is folded into wv on the host, so the score-value
                    # product lands directly in output-channel space.
                    v_sb, st_sb = state
                    for dm in range(C3):
                        dsl = slice(dm * 128, (dm + 1) * 128)
                        o_ps = psp.tile([128, N], F32, name="o_ps", tag="ps")
                        for hlf in range(2):
                            sl = slice(hlf * HALF, (hlf + 1) * HALF)
                            for kc in range(HEADS):
                                MM(o_ps[:, sl], v_sb[kc][:, dsl], st_sb[kc][:, sl],
                                   start=(kc == 0), stop=(kc == HEADS - 1))
                        nc.vector.tensor_add(Y[dm][:], o_ps[:], Y[dm][:])

                state = qkvst(0)
                for h in range(1, HEADS):
                    nxt = qkvst(h)
                    oproj(state)
                    state = nxt
                oproj(state)
                return Y

            def peg(x_tiles):
                """Depthwise 3x3 SAME conv + bias (fp32 in/out)."""
                out = []
                for c in range(C3):
                    acc = xp.tile([128, N], F32, name="peg_acc", tag="x")
                    nc.scalar.activation(
                        acc[:], x_tiles[c][:], AF.Identity,
                        bias=pegb_sb[:, c:c + 1], scale=pegw_sb[:, c, 4:5])
                    a3d = acc[:].rearrange("p (h w) -> p h w", w=W)
                    x3d = x_tiles[c][:].rearrange("p (h w) -> p h w", w=W)
                    eng = nc.vector
                    for dy in (-1, 0, 1):
                        for dx in (-1, 0, 1):
                            if dy == 0 and dx == 0:
                                continue
                            tap = 3 * (dy + 1) + (dx + 1)
                            oh = slice(max(0, -dy), H - max(0, dy))
                            ow = slice(max(0, -dx), W - max(0, dx))
                            ih = slice(max(0, dy), H + min(0, dy))
                            iw = slice(max(0, dx), W + min(0, dx))
                            eng.scalar_tensor_tensor(
                                a3d[:, oh, ow], x3d[:, ih, iw],
                                pegw_sb[:, c, tap:tap + 1], a3d[:, oh, ow],
                                op0=OP.mult, op1=OP.add)
                    out.append(acc)
                return out

            x0 = [xin[:, c, :] for c in range(C3)]

            with (
                tc.tile_pool(name="wq", bufs=4) as wq_p,
                tc.tile_pool(name="wv", bufs=3) as wv_p,
                tc.tile_pool(name="st", bufs=20) as st_p,
                tc.tile_pool(name="v", bufs=20) as v_p,
                tc.tile_pool(name="qk", bufs=6) as qk_p,
                tc.tile_pool(name="bvb", bufs=2) as bvb_p,
            ):
                pools = (wq_p, wv_p, st_p, v_p, qk_p, bvb_p)
                x1 = mha(1, x0, pools)
                x2 = peg(x1)
                x3 = layer_norm(x2, "ln1", BF16, xb)
                x4 = mha(2, x3, pools)

            with tc.tile_pool(name="mlp", bufs=1) as mlp_p, \
                 tc.tile_pool(name="hid", bufs=6) as hid_p:
                w1_sb = mlp_p.tile([128, C3, 768], BF16, name="w1", tag="w1")
                nc.sync.dma_start(w1_sb[:], w1_ext[:])
                w2_sb = mlp_p.tile([128, H6, DIM], BF16, name="w2", tag="w2")
                nc.sync.dma_start(w2_sb[:], w2_ext[:])

                hn = layer_norm(x4, "mlpln", BF16, xb)
                u_sb = []
                for dm in range(C3):
                    u = xp.tile([128, N], F32, name="u", tag="x")
                    nc.vector.tensor_scalar(
                        u[:], x4[dm][:], a3_sb[:, dm:dm + 1], b2_sb[:, dm:dm + 1],
                        op0=OP.mult, op1=OP.add)
                    u_sb.append(u)
                hid = []
                for ht in range(H6):
                    hsl = slice(ht * 128, (ht + 1) * 128)
                    hd_ps = psp.tile([128, N], F32, name="hd_ps", tag="ps")
                    for hlf in range(2):
                        sl = slice(hlf * HALF, (hlf + 1) * HALF)
                        for c in range(C3):
                            MM(hd_ps[:, sl], w1_sb[:, c, hsl], hn[c][:, sl],
                               start=(c == 0), stop=(c == C3 - 1))
                    hg = hid_p.tile([128, N], BF16, name="hg", tag="hid")
                    nc.scalar.activation(hg[:], hd_ps[:], AF.Gelu,
                                         bias=b1_sb[:, ht:ht + 1])
                    hid.append(hg)
                x5 = []
                for dm in range(C3):
                    dsl = slice(dm * 128, (dm + 1) * 128)
                    o2_ps = psp.tile([128, N], F32, name="o2_ps", tag="ps")
                    for hlf in range(2):
                        sl = slice(hlf * HALF, (hlf + 1) * HALF)
                        for ht in range(H6):
                            MM(o2_ps[:, sl], w2_sb[:, ht, dsl], hid[ht][:, sl],
                               start=(ht == 0), stop=(ht == H6 - 1))
                    y = xp.tile([128, N], F32, name="x5t", tag="x")
                    nc.vector.tensor_add(y[:], o2_ps[:], u_sb[dm][:])
                    x5.append(y)

                yout = layer_norm(x5, "ln2", F32, xp)
                for c in range(C3):
                    nc.sync.dma_start(out_ext[c], yout[c][:])

    nc.compile()
    return nc


def _prep_weights(inputs):
    """Host-side reshapes into SBUF-tile-friendly layouts."""
    g = {k: np.ascontiguousarray(np.asarray(v, dtype=np.float32))
         for k, v in inputs.items()}
    s = 1.0 / math.sqrt(DK)
    bf = ml_dtypes.bfloat16
    m = {}
    for i in (1, 2):
        wq = g[f"wq{i}"] * s                      # fold score scale into Q
        m[f"wq{i}"] = wq.reshape(HEADS, C3, 128, DK).transpose(0, 2, 1, 3).astype(bf)
        m[f"wk{i}"] = g[f"wk{i}"].reshape(HEADS, C3, 128, DK).transpose(0, 2, 1, 3).astype(bf)
        wp = g[f"wp{i}"].reshape(HEADS, DIM, DIM)          # [h, 384, 384]
        wvp = np.einsum("hdf,hfe->hde", g[f"wv{i}"], wp)   # fold projector
        bvp = np.einsum("hf,hfe->he", g[f"bv{i}"], wp)
        m[f"wv{i}"] = wvp.reshape(HEADS, C3, 128, DIM).transpose(0, 2, 1, 3).astype(bf)
        m[f"bv{i}"] = bvp.astype(bf)              # [8, 384]
    m["mlp_w1"] = g["mlp_w1"].reshape(C3, 128, 768).transpose(1, 0, 2).astype(bf)
    m["mlp_w2"] = g["mlp_w2"].reshape(H6, 128, DIM).transpose(1, 0, 2).astype(bf)

    def col3(v):
        return np.asarray(v, np.float32).reshape(DIM).reshape(C3, 128).T

    cf = np.zeros((128, 96), np.float32)
    for j, k in enumerate(("ln1", "mlpln", "ln2")):
        cf[:, 3 * j:3 * j + 3] = col3(g[f"{k}_b"])
    cf[:, 9:12] = col3(g["a1"]); cf[:, 12:15] = col3(g["a2"])
    cf[:, 15:18] = col3(g["a3"])
    cf[:, 18:21] = col3(g["bp1"]); cf[:, 21:24] = col3(g["bp2"])
    cf[:, 24:51] = g["peg_w"].reshape(DIM, 9).reshape(C3, 128, 9).transpose(
        1, 0, 2).reshape(128, 27)
    cf[:, 51:54] = col3(g["peg_b"])
    cf[:, 54:60] = g["mlp_b1"].reshape(H6, 128).T
    cf[:, 60:63] = col3(g["mlp_b2"])
    cf[0:DK, 63:71] = (g["bq1"] * s).T
    cf[0:DK, 71:79] = g["bk1"].T
    cf[0:DK, 79:87] = (g["bq2"] * s).T
    cf[0:DK, 87:95] = g["bk2"].T
    m["constf"] = cf
    cg = np.concatenate([g[f"{k}_g"].reshape(DIM)
                         for k in ("ln1", "mlpln", "ln2")]).reshape(1, 3 * DIM)
    m["constg"] = cg.astype(bf)
    m = {k: np.ascontiguousarray(v) for k, v in m.items()}
    return m, g


_NC_CACHE = None


def kernel(**inputs) -> np.ndarray:
    global LAST_EXEC_TIME_NS, _NC_CACHE
    weights, g = _prep_weights(inputs)
    bf = ml_dtypes.bfloat16
    dec = g["decoder"].reshape(B, C3, 128, N).transpose(0, 2, 1, 3).astype(bf)

    if _NC_CACHE is None:
        _NC_CACHE = build_nc()
    nc = _NC_CACHE

    in_maps = []
    for b in range(B):
        im = {"x": np.ascontiguousarray(dec[b])}
        im.update(weights)
        in_maps.append(im)

    trace = bool(int(os.environ.get("KERNEL_TRACE", "0")))
    if trace:
        trace = _install_profile_hook()
    res = run_bass_kernel_spmd(nc, in_maps, core_ids=list(range(B)), trace=trace)
    LAST_EXEC_TIME_NS = res.exec_time_ns

    out = np.stack([np.asarray(res.results[b]["out"]) for b in range(B)], axis=0)
    return np.ascontiguousarray(
        out.reshape(B, DIM, H, W).astype(np.float32))


def _install_profile_hook():
    """Register the axon NTFF profiling hook this image's antenv lacks."""
    import sys
    import types
    try:
        from concourse import bass_utils as _bu
        _bu.upload_artifacts = lambda tmpdir: tmpdir
        try:
            import antenv.axon_hooks  # noqa: F401
            return True
        except ImportError:
            pass
        import antenv
        mod = types.ModuleType("antenv.axon_hooks")
        state = {"hook": None}
        mod.set_axon_ntff_profile_hook = lambda h: state.__setitem__("hook", h)
        mod.get_axon_ntff_profile_hook = lambda: state["hook"]
        sys.modules["antenv.axon_hooks"] = mod
        antenv.axon_hooks = mod
        from trn_agent_boot.trn_boot import _ntff_profile_via_ctypes
        mod.set_axon_ntff_profile_hook(
            _ntff_profile_via_ctypes("/opt/axon/libaxon_pjrt.so"))
        return True
    except Exception:
        return False
